# revision 6
# baseline (speedup 1.0000x reference)
"""Deformable Conv3D kernel for TRN2 — gather formulation (ap_gather on GPSIMD).

Per 2D image n (12 = B*D images): offsets via 3x3 conv on PE (bf16); bilinear
corner positions become int16 flat indices into the fp32 x slab; the four
corners are fetched with GPSIMD ap_gather (one gather per 3-tap batch per
corner, corner offsets folded into shifted input windows); corner weights
(1∓fy)(1∓fx) are replicated 36→128 partitions by one-hot matmuls; DVE
multiplies weight×corner and block-diag grouped matmuls accumulate the conv
in PSUM; instance-norm stats all-reduced across cores; exact-GELU on ACT.

ap_gather's per-16-partition wrapped index layout means gathered planes come
out in a fixed permutation sigma(m) = 112*(m%16) + m//16 of the flat padded
grid; weights are built sigma-ordered via strided APs and the final PSUM
eviction un-permutes, so no data ever needs re-ordering on its own.

Sharding: 24 half-image jobs (28 rows), 3 per core, core c owns jobs 3c..3c+2
(all in batch c//4, so norm groups are [[0..3],[4..7]]).
"""
import os
os.environ.setdefault("JAX_PLATFORMS", "cpu")
from contextlib import ExitStack

import numpy as np
import ml_dtypes

import concourse.bass as bass
import concourse.tile as tile
from concourse import mybir
from concourse._compat import with_exitstack

AF = mybir.ActivationFunctionType
ALU = mybir.AluOpType
FP32 = mybir.dt.float32
BF16 = mybir.dt.bfloat16
I16 = mybir.dt.int16
NPBF16 = ml_dtypes.bfloat16

G, K2, CG, COUT = 4, 9, 32, 128
B, C, D, H, W = 2, 128, 6, 56, 56
NIMG = B * D
EPS = 1e-5

WPAD = 64           # padded row pitch
ROWS = 36           # slab rows: image rows r0-4 .. r0+31 (rows 0, 35 = guards)
SLAB = ROWS * WPAD  # 2304
XPADW = SLAB + 66   # slab + tail so corner windows stay in-bounds
OGBASE = 4 * WPAD   # out-grid flat origin (buffer row 4, col 0)
FD = 28 * WPAD      # 1792
CO = 3              # out col w -> buffer col w+CO
NJOB = 3
NCORES = 8
CHUNK = 7 * WPAD    # 448
NCH = 4
VALID = 28 * 56     # 1568
TAPB = ((0, 2), (2, 2), (4, 2), (6, 2), (8, 1))  # tap batches for gathers
CORNER_SHIFTS = (0, 1, WPAD, WPAD + 1)   # 00, 01, 10, 11
DC_NJ = int(os.environ.get("DC_NJ", str(NJOB)))
DC_STAGE = int(os.environ.get("DC_STAGE", "3"))

IN_SHAPES = {
    "xslab": (NJOB, C, ROWS, WPAD),
    "offw_t": (C, K2 * 72),
    "offb_p": (36, 2),
    "wblk": (128, K2 * 128),
    "convb": (128, 1),
    "sel": (36, K2 * 128),
    "pconst": (36, FD),
    "sig16": (48, 112),
    "siginv16": (128, 112),
}
IN_DTYPES = {
    "xslab": FP32,
    "offw_t": BF16,
    "offb_p": FP32,
    "wblk": BF16,
    "convb": FP32,
    "sel": BF16,
    "pconst": FP32,
    "sig16": I16,
    "siginv16": I16,
}
OUT_SHAPES = {"y": (NJOB, 128, 28, 56)}


def taps():
    return [(k, k // 3 - 1, k % 3 - 1) for k in range(K2)]


def host_prep(inputs):
    """Per-core input maps. Pure layout/permutation work."""
    x = np.ascontiguousarray(np.asarray(inputs["x"], np.float32))
    offset_w = np.asarray(inputs["offset_w"], np.float32)
    offset_b = np.asarray(inputs["offset_b"], np.float32)
    conv_w = np.asarray(inputs["conv_w"], np.float32)
    conv_b = np.asarray(inputs["conv_b"], np.float32)

    x2d = x.transpose(0, 2, 1, 3, 4).reshape(NIMG, C, H, W)

    offw_t = np.zeros((K2, C, 72), np.float32)
    offb_p = np.zeros((36, 2), np.float32)
    for isx in range(2):
        for g in range(G):
            for k in range(K2):
                j2 = 36 * isx + 9 * g + k
                oc = 2 * (9 * g + k) + isx
                # +2 folded in so the conv eviction directly yields off+2
                offb_p[9 * g + k, isx] = offset_b[oc] + 2.0
                for kk, ky, kx in taps():
                    offw_t[kk, :, j2] = offset_w[oc, :, ky + 1, kx + 1]

    wblk = np.zeros((K2, 128, 128), np.float32)
    for kk, ky, kx in taps():
        for g in range(G):
            wblk[kk, 32 * g : 32 * g + 32, 32 * g : 32 * g + 32] = conv_w[
                32 * g : 32 * g + 32, :, ky + 1, kx + 1
            ].T
    convb = conv_b.reshape(128, 1).astype(np.float32)

    sel = np.zeros((K2, 36, 128), np.float32)
    for k in range(K2):
        for g in range(G):
            sel[k, 9 * g + k, 32 * g : 32 * g + 32] = 1.0

    # flat-index base: idx = pconst + 64*(floor(off_y)+2) + (floor(off_x)+2)
    # pconst[9g+k, p] = p + 64*(ky+4) + kx - 130  (off planes are on the
    # padded buffer grid, so CO is already inside p)
    pconst = np.zeros((36, FD), np.float32)
    parr = np.arange(FD, dtype=np.float32)
    for g in range(G):
        for k, ky, kx in taps():
            pconst[9 * g + k] = parr + 64 * (ky + 4) + kx - 130

    # constant gather tables: sigma (weights -> gathered m-order) and
    # sigma-inverse (m-order conv -> flat). ap_gather semantics:
    # out[m] = in[idx[m % 16, m // 16]] per 16-partition block.
    sig16 = np.zeros((48, 112), np.int16)
    siginv16 = np.zeros((128, 112), np.int16)
    for m in range(FD):
        p, s = m % 16, m // 16
        sig16[p, s] = 112 * p + s          # sigma(m)
        siginv16[p, s] = 16 * (m % 112) + m // 112  # sigma^-1(m)
    sig16[16:32] = sig16[0:16]
    sig16[32:48] = sig16[0:16]
    for b in range(1, 8):
        siginv16[16 * b : 16 * (b + 1)] = siginv16[0:16]

    in_maps = []
    for c in range(NCORES):
        slab = np.zeros((NJOB, C, ROWS, WPAD), np.float32)
        for j in range(NJOB):
            job = 3 * c + j
            n, r0 = job // 2, 28 * (job % 2)
            for bb in range(ROWS):
                r = r0 + bb - 4
                if 0 <= r < H:
                    slab[j, :, bb, CO : CO + W] = x2d[n, :, r, :]
        in_maps.append(
            {
                "xslab": slab,
                "offw_t": np.ascontiguousarray(
                    offw_t.transpose(1, 0, 2).reshape(C, K2 * 72)
                ).astype(NPBF16),
                "offb_p": offb_p,
                "wblk": np.ascontiguousarray(
                    wblk.transpose(1, 0, 2).reshape(128, K2 * 128)
                ).astype(NPBF16),
                "convb": convb,
                "sel": np.ascontiguousarray(
                    sel.transpose(1, 0, 2).reshape(36, K2 * 128)
                ).astype(NPBF16),
                "pconst": pconst,
                "sig16": sig16,
                "siginv16": siginv16,
            }
        )
    return in_maps


def assemble(outs):
    full = np.zeros((B, COUT, D, H, W), np.float32)
    for c in range(NCORES):
        y = outs[c]["y"]
        for j in range(NJOB):
            job = 3 * c + j
            n, r0 = job // 2, 28 * (job % 2)
            bidx, d = n // D, n % D
            full[bidx, :, d, r0 : r0 + 28, :] = y[j]
    return full


def _sigma_dst(ap_1792):
    """View a contiguous 1792-col slice in sigma order [q, ss(112), p(16)]:
    element (ss, p) is m = 16*ss + p."""
    return ap_1792.rearrange("q (ss p) -> q ss p", p=16)


def _sigma_src(ap_1792):
    """sigma-source view: element (ss, p) is flat position 112*p + ss."""
    return ap_1792.rearrange("q (p ss) -> q ss p", p=16)


@with_exitstack
def dc_kernel(ctx: ExitStack, tc: tile.TileContext, outs, ins, n_cores=8):
    nc = tc.nc
    y_out = outs["y"]  # dram [NJOB, 128, 28, 56] f32
    xslab_d, offwt_d, offb_d = ins["xslab"], ins["offw_t"], ins["offb_p"]
    wblk_d, convb_d, sel_d = ins["wblk"], ins["convb"], ins["sel"]
    pconst_d = ins["pconst"]

    const = ctx.enter_context(tc.tile_pool(name="const", bufs=1))
    pln = ctx.enter_context(tc.tile_pool(name="pln", bufs=1))
    pool = ctx.enter_context(tc.tile_pool(name="work", bufs=1))
    wrep_pool = ctx.enter_context(tc.tile_pool(name="wrepp", bufs=8))
    gout_pool = ctx.enter_context(tc.tile_pool(name="goutp", bufs=2))
    xw_pool = ctx.enter_context(tc.tile_pool(name="xw", bufs=2))
    ps_rep = ctx.enter_context(tc.tile_pool(name="ps_rep", bufs=2, space="PSUM"))
    ps_out = ctx.enter_context(tc.tile_pool(name="ps_out", bufs=1, space="PSUM"))
    dram = ctx.enter_context(tc.tile_pool(name="dramp", bufs=1, space="DRAM"))

    # ---- constants
    offw_t = const.tile([C, K2 * 72], BF16)
    nc.sync.dma_start(offw_t[:], offwt_d[:])
    offb = const.tile([36, 2], FP32)
    nc.sync.dma_start(offb[:], offb_d[:])
    wblk = const.tile([128, K2 * 128], BF16)
    nc.sync.dma_start(wblk[:], wblk_d[:])
    convb = const.tile([128, 1], FP32)
    nc.sync.dma_start(convb[:], convb_d[:])
    sel = const.tile([36, K2 * 128], BF16)
    nc.sync.dma_start(sel[:], sel_d[:])
    pconst = const.tile([36, FD], FP32)
    nc.sync.dma_start(pconst[:], pconst_d[:])
    sig16 = const.tile([48, 112], I16)
    nc.sync.dma_start(sig16[:], ins["sig16"][:])
    siginv16 = const.tile([128, 112], I16)
    nc.sync.dma_start(siginv16[:], ins["siginv16"][:])
    # w4 plane buffers: rows 36..47 must be defined for the 48-channel
    # sigma-gathers; zero them once (rows 0..35 rewritten every job).
    w4a = const.tile([48, FD], FP32)
    w4b = const.tile([48, FD], FP32)
    nc.vector.memset(w4a[:], 0.0)
    nc.vector.memset(w4b[:], 0.0)

    convout = const.tile([128, NJOB * FD], FP32)
    stats_s = const.tile([128, NJOB], FP32)
    stats_q = const.tile([128, NJOB], FP32)

    evict_idx = 0

    for j in range(DC_NJ):
        xpad32 = pool.tile([C, XPADW], FP32, tag="xpad32")
        nc.sync.dma_start(xpad32[:, 0:SLAB],
                          xslab_d[j].rearrange("c r w -> c (r w)"))
        xpadbf = pool.tile([C, SLAB], BF16, tag="xpadbf")
        nc.scalar.activation(xpadbf[:], xpad32[:, 0:SLAB], AF.Copy)

        # ---- offset conv -> t_y / t_x = off+2 in (0,4), [36, FD]
        t_y = pool.tile([36, FD], FP32, tag="t_y")
        t_x = pool.tile([36, FD], FP32, tag="t_x")
        for m in range(NCH):
            for isx, odst in ((0, t_y), (1, t_x)):
                prep = ps_rep.tile([128, 1024], FP32, tag="prep",
                                   name=f"po_{j}_{m}_{isx}")
                po = prep[0:36, 0:CHUNK]
                for i, (kk, ky, kx) in enumerate(taps()):
                    d0 = OGBASE + ky * WPAD + kx + m * CHUNK
                    nc.tensor.matmul(
                        po,
                        offw_t[:, kk * 72 + isx * 36 : kk * 72 + isx * 36 + 36],
                        xpadbf[:, d0 : d0 + CHUNK],
                        start=(i == 0),
                        stop=(i == K2 - 1),
                    )
                nc.scalar.activation(
                    odst[:, m * CHUNK : (m + 1) * CHUNK], po, AF.Identity,
                    bias=offb[:, isx : isx + 1],
                )

        # ---- floor + frac (rounding-mode safe), fy/fx in [0,1)
        fy = pool.tile([36, FD], FP32, tag="fy")
        fx = pool.tile([36, FD], FP32, tag="fx")
        # w4a/w4b rows 0..35 double as floor scratch until iflat is built
        y0p2 = w4a[0:36, :]
        x0p2 = w4b[0:36, :]
        itmp = pool.tile([36, FD], I16, tag="itmp")
        for tsrc, ip2, frac in ((t_y, y0p2, fy), (t_x, x0p2, fx)):
            nc.vector.tensor_copy(itmp[:], tsrc[:])          # fp32 -> int16
            nc.vector.tensor_copy(ip2, itmp[:])              # int16 -> fp32
            # correction: wherever cast rounded up (ip2 > t), subtract 1
            nc.vector.tensor_tensor(frac[:], tsrc[:], ip2, op=ALU.is_lt)
            nc.vector.tensor_sub(ip2, ip2, frac[:])
            nc.vector.tensor_sub(frac[:], tsrc[:], ip2)

        # ---- flat corner-00 index, int16, then wrap-distribute via DMA
        # (reuses t_y's buffer: t_y is dead once fy/y0p2 exist)
        iflat = pool.tile([36, FD], FP32, tag="t_y")
        nc.vector.scalar_tensor_tensor(
            iflat[:], y0p2, 64.0, x0p2, op0=ALU.mult, op1=ALU.add
        )
        nc.vector.tensor_add(iflat[:], iflat[:], pconst[:])
        iflat16 = pool.tile([36, FD], I16, tag="itmp")
        nc.vector.tensor_copy(iflat16[:], iflat[:])

        # wrap-distribute via a DRAM bounce: store the flat int16 index
        # plane, then per (g,k) load 16 contiguous 112-element runs into
        # the 2x-duplicated 16-partition blocks of idxall.
        ifl_dram = dram.tile([36, FD], I16, name=f"ifl_dram_{j}")
        nc.sync.dma_start(ifl_dram[:], iflat16[:])
        idxall = pool.tile([128, K2 * 112], I16, tag="idxall")
        for g in range(G):
            for k in range(K2):
                srcv = ifl_dram[9 * g + k].rearrange("(i ss) -> i ss", i=16)
                for dup in range(2):
                    p0 = 32 * g + 16 * dup
                    nc.sync.dma_start(
                        idxall[p0 : p0 + 16, k * 112 : (k + 1) * 112], srcv)

        # ---- corner weights, flat [36,FD] then sigma-permuted via constant
        # -index ap_gather (48 channels) and converted to bf16.
        # w11 = fy*fx; w10 = fy - w11; w01 = fx - w11; w00 = 1 - fy - w01.
        w4s = pool.tile([36, 4 * FD], BF16, tag="w4s")

        def sig_plane(ci, plane48):
            wsf = pln.tile([48, FD], FP32, tag="w4sf", bufs=2)
            nc.gpsimd.ap_gather(
                wsf[:], plane48[:], sig16[:],
                channels=48, num_elems=FD, d=1, num_idxs=FD,
            )
            nc.scalar.activation(
                w4s[:, ci * FD : (ci + 1) * FD], wsf[0:36, :], AF.Copy)

        nc.vector.tensor_mul(w4a[0:36, :], fy[:], fx[:])          # w11
        sig_plane(3, w4a)
        nc.vector.tensor_sub(w4b[0:36, :], fy[:], w4a[0:36, :])   # w10
        sig_plane(2, w4b)
        nc.vector.tensor_sub(w4b[0:36, :], fx[:], w4a[0:36, :])   # w01
        sig_plane(1, w4b)
        nc.vector.tensor_add(w4a[0:36, :], fy[:], w4b[0:36, :])   # fy+w01
        nc.scalar.activation(w4a[0:36, :], w4a[0:36, :], AF.Identity,
                             bias=1.0, scale=-1.0)                # w00
        sig_plane(0, w4a)

        if DC_STAGE < 2:
            continue
        # ---- main loop: per tap batch, per corner: gather + weight + matmul
        pout = []
        for m in range(NCH):
            pt = ps_out.tile([128, CHUNK], FP32, tag=f"pout{m}", name=f"pout{m}_{j}")
            pout.append(pt)
        first = True
        for k0, nt in TAPB:
            # corner-weight replication for this batch (independent of gathers)
            wreps = {}
            for tt in range(nt):
                k = k0 + tt
                for ci in range(4):
                    wrep = wrep_pool.tile([128, FD], BF16, tag="wrep")
                    for half in range(2):
                        prep = ps_rep.tile([128, 1024], FP32, tag="prep",
                                           name=f"wr_{j}_{k}_{ci}_{half}")
                        lo = half * 896
                        for t in range(2):
                            nc.tensor.matmul(
                                prep[:, t * 512 : t * 512 + CHUNK],
                                sel[:, k * 128 : (k + 1) * 128],
                                w4s[:, ci * FD + lo + t * CHUNK
                                    : ci * FD + lo + (t + 1) * CHUNK],
                                start=True,
                                stop=True,
                            )
                        src = prep[:].rearrange("p (b c) -> p b c", b=2)[:, :, 0:CHUNK]
                        dst = wrep[:, lo : lo + 896].rearrange(
                            "p (b c) -> p b c", b=2)
                        if evict_idx % 4 == 3:
                            nc.vector.tensor_copy(dst, src)
                        else:
                            nc.scalar.activation(dst, src, AF.Copy)
                        evict_idx += 1
                    wreps[(tt, ci)] = wrep

            for ci, dv in enumerate(CORNER_SHIFTS):
                gout = gout_pool.tile([128, nt * FD], FP32, tag="gout")
                nc.gpsimd.ap_gather(
                    gout[:], xpad32[:, dv : dv + SLAB],
                    idxall[:, k0 * 112 : (k0 + nt) * 112],
                    channels=128, num_elems=SLAB, d=1, num_idxs=nt * FD,
                )
                for tt in range(nt):
                    k = k0 + tt
                    xw = xw_pool.tile([128, FD], BF16, tag="xw")
                    nc.vector.tensor_mul(
                        xw[:], wreps[(tt, ci)][:],
                        gout[:, tt * FD : (tt + 1) * FD],
                    )
                    last = (k0 + nt == K2) and ci == 3 and tt == nt - 1
                    for m in range(NCH):
                        nc.tensor.matmul(
                            pout[m][:],
                            wblk[:, k * 128 : (k + 1) * 128],
                            xw[:, m * CHUNK : (m + 1) * CHUNK],
                            start=first,
                            stop=last,
                        )
                    first = False

        # ---- evict + bias contiguously (still m-order), then un-permute
        # with a sigma^-1 constant gather into convout.
        convm = pool.tile([128, FD], FP32, tag="convm")
        for m in range(NCH):
            nc.scalar.activation(convm[:, m * CHUNK : (m + 1) * CHUNK],
                                 pout[m][:], AF.Identity, bias=convb[:])
        cj = convout[:, j * FD : (j + 1) * FD]
        nc.gpsimd.ap_gather(cj, convm[:], siginv16[:],
                            channels=128, num_elems=FD, d=1, num_idxs=FD)

        # ---- stats over valid cols only
        cjv = cj.rearrange("q (r w) -> q r w", r=28)[:, :, CO : CO + 56]
        nc.vector.tensor_reduce(stats_s[:, j : j + 1], cjv,
                                axis=mybir.AxisListType.XY, op=ALU.add)
        scr = pool.tile([128, VALID], FP32, tag="scrfin")
        nc.scalar.activation(scr[:], cjv, AF.Square)
        nc.vector.tensor_reduce(stats_q[:, j : j + 1], scr[:],
                                axis=mybir.AxisListType.X, op=ALU.add)

    # ---- norm stats all-reduce
    if DC_STAGE < 3:
        for j in range(DC_NJ):
            fin = pool.tile([128, VALID], FP32, tag="scrfin")
            nc.vector.memset(fin[:], 0.0)
            nc.sync.dma_start(y_out[j].rearrange("c r w -> c (r w)"), fin[:])
        return
    red = const.tile([128, 2], FP32)
    nc.vector.tensor_reduce(red[:, 0:1], stats_s[:, 0:DC_NJ],
                            axis=mybir.AxisListType.X, op=ALU.add)
    nc.vector.tensor_reduce(red[:, 1:2], stats_q[:, 0:DC_NJ],
                            axis=mybir.AxisListType.X, op=ALU.add)

    if n_cores > 4:
        groups = [[0, 1, 2, 3], [4, 5, 6, 7]]
    else:
        groups = [list(range(n_cores))]
    bounce_in = dram.tile([128, 2], FP32)
    bounce_out = dram.tile([128, 2], FP32)
    nc.gpsimd.dma_start(bounce_in[:], red[:])
    nc.gpsimd.collective_compute(
        "AllReduce", ALU.add, replica_groups=groups,
        ins=[bounce_in.opt()], outs=[bounce_out.opt()],
    )
    allred = const.tile([128, 2], FP32)
    nc.gpsimd.dma_start(allred[:], bounce_out[:])

    NTOT = float(len(groups[0]) * NJOB * VALID)
    mom = const.tile([128, 4], FP32)
    nc.vector.tensor_scalar_mul(mom[:, 0:1], allred[:, 0:1], 1.0 / NTOT)
    nc.vector.tensor_scalar_mul(mom[:, 1:2], allred[:, 1:2], 1.0 / NTOT)
    msq = const.tile([128, 1], FP32)
    nc.vector.tensor_mul(msq[:], mom[:, 0:1], mom[:, 0:1])
    nc.vector.tensor_sub(mom[:, 2:3], mom[:, 1:2], msq[:])
    nc.vector.tensor_scalar_add(mom[:, 2:3], mom[:, 2:3], EPS)
    nc.scalar.activation(mom[:, 3:4], mom[:, 2:3], AF.Sqrt)
    scale = const.tile([128, 1], FP32)
    nc.vector.reciprocal(scale[:], mom[:, 3:4])
    nbias = const.tile([128, 1], FP32)
    nc.vector.tensor_mul(nbias[:], mom[:, 0:1], scale[:])
    nc.vector.tensor_scalar_mul(nbias[:], nbias[:], -1.0)

    # ---- GELU epilogue + store
    for j in range(DC_NJ):
        fin = pool.tile([128, VALID], FP32, tag="scrfin")
        src = convout[:, j * FD : (j + 1) * FD].rearrange(
            "q (r w) -> q r w", r=28)[:, :, CO : CO + 56]
        nc.scalar.activation(fin[:], src, AF.Gelu, bias=nbias[:], scale=scale[:])
        nc.sync.dma_start(y_out[j].rearrange("c r w -> c (r w)"), fin[:])



# ---------------- self-contained runner ----------------
import concourse.bass_utils as _bass_utils
from concourse import bacc as _bacc

_NC_CACHE = {}


def _build_nc(n_cores=8):
    if n_cores in _NC_CACHE:
        return _NC_CACHE[n_cores]
    nc = _bacc.Bacc(
        "TRN2", target_bir_lowering=False, debug=False,
        enable_asserts=False, num_devices=n_cores,
    )
    ins = {
        name: nc.dram_tensor(name, list(shp), IN_DTYPES[name],
                             kind="ExternalInput").ap()
        for name, shp in IN_SHAPES.items()
    }
    outs = {
        "y": nc.dram_tensor("y", list(OUT_SHAPES["y"]), FP32,
                            kind="ExternalOutput").ap()
    }
    with tile.TileContext(nc) as tc:
        dc_kernel(tc, outs, ins, n_cores=n_cores)
    nc.compile()
    _NC_CACHE[n_cores] = nc
    return nc


_EXEC_CACHE = {}


def _build_exec(n_cores=8):
    """Cached sharded executable (run_bass_via_pjrt retraces per call; we don't)."""
    if n_cores in _EXEC_CACHE:
        return _EXEC_CACHE[n_cores]
    import jax
    import concourse.mybir as _mybir
    from jax.experimental.shard_map import shard_map
    from jax.sharding import Mesh, PartitionSpec
    from concourse.bass2jax import (
        _bass_exec_p, install_neuronx_cc_hook, partition_id_tensor,
    )

    nc = _build_nc(n_cores)
    install_neuronx_cc_hook()
    partition_name = nc.partition_id_tensor.name if nc.partition_id_tensor else None
    in_names, out_names, out_avals, zero_outs = [], [], [], []
    for alloc in nc.m.functions[0].allocations:
        if not isinstance(alloc, _mybir.MemoryLocationSet):
            continue
        name = alloc.memorylocations[0].name
        if alloc.kind == "ExternalInput":
            if name != partition_name:
                in_names.append(name)
        elif alloc.kind == "ExternalOutput":
            shape = tuple(alloc.tensor_shape)
            dtype = _mybir.dt.np(alloc.dtype)
            out_names.append(name)
            out_avals.append(jax.core.ShapedArray(shape, dtype))
            zero_outs.append(np.zeros(shape, dtype))
    n_params, n_outs = len(in_names), len(out_avals)
    all_names = list(in_names) + list(out_names)
    if partition_name is not None:
        all_names.append(partition_name)
    donate = tuple(range(n_params, n_params + n_outs))

    def _body(*args):
        operands = list(args)
        if partition_name is not None:
            operands.append(partition_id_tensor())
        outs = _bass_exec_p.bind(
            *operands,
            out_avals=tuple(out_avals),
            in_names=tuple(all_names),
            out_names=tuple(out_names),
            lowering_input_output_aliases=(),
            sim_require_finite=True,
            sim_require_nnan=True,
            nc=nc,
        )
        return tuple(outs)

    devices = jax.devices()[:n_cores]
    mesh = Mesh(np.asarray(devices), ("core",))
    in_specs = (PartitionSpec("core"),) * (n_params + n_outs)
    out_specs = (PartitionSpec("core"),) * n_outs
    sharded = jax.jit(
        shard_map(_body, mesh=mesh, in_specs=in_specs, out_specs=out_specs,
                  check_rep=False),
        donate_argnums=donate, keep_unused=True,
    )
    ctx = (sharded, in_names, out_names, out_avals, zero_outs, n_cores)
    _EXEC_CACHE[n_cores] = ctx
    return ctx


def _execute(in_maps):
    sharded, in_names, out_names, out_avals, zero_outs, n_cores = _build_exec(8)
    concat_in = [
        np.concatenate([in_maps[c][name] for c in range(n_cores)], axis=0)
        for name in in_names
    ]
    concat_zero = [
        np.zeros((n_cores * z.shape[0], *z.shape[1:]), z.dtype) for z in zero_outs
    ]
    out_arrs = sharded(*concat_in, *concat_zero)
    return [
        {
            name: np.asarray(out_arrs[i]).reshape(n_cores, *out_avals[i].shape)[c]
            for i, name in enumerate(out_names)
        }
        for c in range(n_cores)
    ]


def run(inputs, trace=False):
    in_maps = host_prep(inputs)
    results = _execute(in_maps)
    return assemble(results), results


def kernel(**inputs):
    return run(inputs)[0]
